# revision 44
# baseline (speedup 1.0000x reference)
"""Trainium2 Bass kernel for a LoRA-MoE layer (gate top-2 softmax routing +
dense base linear + per-expert low-rank adapters), SPMD across 8 NeuronCores.

Math (per token t):
    logits = x @ gate_w.T                      # [E]
    top-2 softmax over logits -> dense w[E] (0 for non-selected)
    out = x @ base_w.T + base_b
        + SCALING * sum_e w[e] * (x @ lora_A[e].T) @ lora_B[e].T

Key identity used: with w folded into the rank-space activations,
    lora_out = (low * w_rep) @ B_all.T,  low = x @ A_all.T   (A_all: [E*R, D])
so the whole MoE-LoRA is two dense matmuls + tiny gating vector math.
Top-2 softmax via sigmoid: w(g) = sigmoid(2g - m1 - m2) for g in {m1, m2}.

Sharding: 8-way over tokens (T=512/core), base W replicated (full O=4096
per core).  All matmul operands bf16 (PSUM accumulates fp32); output bf16,
host converts to fp32.  No collectives.

Layout per core (contraction dim on partitions):
    out.T[o, t] = sum_d W[o, d] * x.T[d, t]    (x.T moving, W tiles stationary)

Schedule: phase A (low+gate, 64 matmuls) -> gating math on DVE/gpsimd/ACT
overlapped under the first base-W chains; each out-tile's lora-B matmul is
appended to its (still open) PSUM chain 3 out-tiles later so the gating
latency never stalls the PE.
"""

import numpy as np
import ml_dtypes

import concourse.bass as bass
import concourse.bass_isa as bass_isa
import concourse.mybir as mybir
import concourse.tile as tile
from concourse import bacc
from concourse.bass_utils import run_bass_kernel_spmd

F32 = mybir.dt.float32
BF16 = mybir.dt.bfloat16
NP_BF16 = ml_dtypes.bfloat16

# Problem constants
B, S, D, O = 2, 2048, 4096, 4096
E, R = 8, 16
ER = E * R  # 128
SCALING = 32.0 / 16.0

# Sharding: 8 token groups, base W replicated
N_CORES = 8
T = (B * S) // N_CORES  # 512 tokens per core
KT = D // 128           # 32 contraction tiles
OTN = O // 128          # 32 out tiles per core
XSPLIT = (2, 2) + (4,) * 7  # x DMA chunk sizes in k-tiles
GKT = 4                 # PE gate/low interleave group size in k-tiles
ASPLIT = (4, 12, 16)    # aT chunk sizes in k-tiles
W_SYNC = 6              # first W tiles ride the sync queue behind x
W_BUFS = 6              # W prefetch depth
WARMUP_MM = 9           # dummy matmuls to ramp the PE p-state before phase A


def build_body(nc, tc, tensors):
    xT, wT, aT, gT, bT, bias2, Rm, out = tensors
    OP = mybir.AluOpType

    with (
        tc.tile_pool(name="xp", bufs=1) as xp,
        tc.tile_pool(name="wp", bufs=W_BUFS) as wp,
        tc.tile_pool(name="cst", bufs=1) as cst,
        tc.tile_pool(name="gw", bufs=1) as gw,
        tc.tile_pool(name="outp", bufs=4) as outp,
        tc.tile_pool(name="psA", bufs=1, space="PSUM") as psA,
        tc.tile_pool(name="psB", bufs=5, space="PSUM") as psB,
    ):
        # ---- PE warm-up: dummy matmuls on a memset tile ramp the PE to
        # its max p-state while the first input DMAs are in flight ----
        warm_sb = gw.tile([128, T], BF16, tag="warm")
        nc.vector.memset(warm_sb[:], 0.0)
        wrep_ps = psA.tile([ER, T], F32, tag="wrep")
        for _ in range(WARMUP_MM):
            nc.tensor.matmul(wrep_ps[:], lhsT=warm_sb[:, :128], rhs=warm_sb[:],
                             start=True, stop=True)

        # ---- tiny constants on the (otherwise idle) gpsimd queue ----
        Rm_sb = cst.tile([E, ER], BF16)
        nc.gpsimd.dma_start(out=Rm_sb[:], in_=Rm[:])
        bias_sb = cst.tile([128, OTN], F32)
        nc.gpsimd.dma_start(out=bias_sb[:], in_=bias2[:])

        # ---- phase-A inputs ride the two fast queues (sync + vector) in
        # consumption order; W0 snuck in before the last x chunks ----
        a_ch = []
        x_ch = []
        w_tiles = {}

        gT_sb = cst.tile([128, KT, E], BF16)
        nc.sync.dma_start(out=gT_sb[:], in_=gT[:])
        # aT arrives in small chunks woven between x chunks: each aT chunk
        # lands just before the (one-chunk-lagged) low pass needs it, and
        # nothing bulky ever delays x
        a_starts = np.cumsum([0] + list(ASPLIT[:-1]))
        a_at = {0: 0, 1: 1, 2: 3}            # aT chunk i sits before x chunk a_at[i]
        x_base = 0
        for c, nk in enumerate(XSPLIT):
            for i, xpos in a_at.items():
                if xpos == c:
                    ac = cst.tile([128, ASPLIT[i], 128], BF16, name=f"a{i}")
                    st = int(a_starts[i])
                    nc.sync.dma_start(out=ac[:], in_=aT[:, st:st + ASPLIT[i], :])
                    a_ch.append((st, ac))
            xc = xp.tile([128, nk, T], BF16, name=f"x{c}")
            if c == len(XSPLIT) - 1:
                wq = wp.tile([128, KT, 128], BF16, tag="w", name="w0")
                nc.sync.dma_start(out=wq[:], in_=wT[:, 0])
                w_tiles[0] = wq
            nc.sync.dma_start(out=xc[:], in_=xT[:, x_base:x_base + nk, :])
            x_ch.append((x_base, xc))
            x_base += nk
        for ot in range(1, W_SYNC):
            wq = wp.tile([128, KT, 128], BF16, tag="w", name=f"w{ot}")
            nc.sync.dma_start(out=wq[:], in_=wT[:, ot])
            w_tiles[ot] = wq
        # bT (lora-B weights, 1MB, first needed at finish(0) ~base(3)):
        # emitted at the back of the sync queue — on gpsimd/scalar the
        # scheduler hoists it into the critical x window
        bT_sb = cst.tile([ER, O], BF16)
        nc.sync.dma_start(out=bT_sb[:], in_=bT[:])

        def x_k(k):
            for base, xc_ in reversed(x_ch):
                if k >= base:
                    return xc_[:, k - base, :]

        def a_k(k):
            for base, ac_ in reversed(a_ch):
                if k >= base:
                    return ac_[:, k - base, :]

        # ---- phase A: gate.T = g^T @ x.T and low.T = A_all.T^T @ x.T,
        # interleaved per x chunk with low lagging gate by one chunk (the
        # lag absorbs x/aT arrival jitter); unroll-4 amortizes the PSUM
        # bank switch ----
        low_ps = psA.tile([ER, T], F32, tag="low")
        gate_ps = psA.tile([E, T], F32, tag="gate")
        NG = KT // GKT
        for c in range(NG + 1):
            if c < NG:
                for k in range(c * GKT, (c + 1) * GKT):
                    nc.tensor.matmul(gate_ps[:], lhsT=gT_sb[:, k, :], rhs=x_k(k),
                                     start=(k == 0), stop=(k == KT - 1))
            if c > 0:
                for k in range((c - 1) * GKT, c * GKT):
                    nc.tensor.matmul(low_ps[:], lhsT=a_k(k), rhs=x_k(k),
                                     start=(k == 0), stop=(k == KT - 1))

        # ---- gating math in [E, t] layout (latency hidden under phase B) --
        g_sb = gw.tile([E, T], F32, tag="gsb")
        nc.vector.tensor_copy(g_sb[:], gate_ps[:])
        m1b = gw.tile([E, T], F32, tag="m1b")
        nc.gpsimd.partition_all_reduce(m1b[:], g_sb[:], channels=E,
                                       reduce_op=bass_isa.ReduceOp.max)
        eq = gw.tile([E, T], F32, tag="eq")
        nc.vector.tensor_tensor(eq[:], g_sb[:], m1b[:], op=OP.is_equal)
        gm = gw.tile([E, T], F32, tag="gm")
        nc.vector.scalar_tensor_tensor(gm[:], in0=eq[:], scalar=-1e30, in1=g_sb[:],
                                       op0=OP.mult, op1=OP.add)
        m2b = gw.tile([E, T], F32, tag="m2b")
        nc.gpsimd.partition_all_reduce(m2b[:], gm[:], channels=E,
                                       reduce_op=bass_isa.ReduceOp.max)
        # top-2 softmax weights: w = sigmoid(2g - m1 - m2) on the two
        # selected rows (exact: sigmoid(m1-m2) and sigmoid(m2-m1))
        ssum = gw.tile([E, T], F32, tag="ssum")
        nc.vector.tensor_tensor(ssum[:], m1b[:], m2b[:], op=OP.add)
        dd = gw.tile([E, T], F32, tag="dd")
        nc.vector.scalar_tensor_tensor(dd[:], in0=g_sb[:], scalar=2.0, in1=ssum[:],
                                       op0=OP.mult, op1=OP.subtract)
        sg = gw.tile([E, T], F32, tag="sg")
        nc.scalar.activation(sg[:], dd[:], mybir.ActivationFunctionType.Sigmoid)
        mask = gw.tile([E, T], F32, tag="mask")
        nc.vector.tensor_tensor(mask[:], g_sb[:], m2b[:], op=OP.is_ge)
        wsc = gw.tile([E, T], BF16, tag="wsc")
        nc.vector.scalar_tensor_tensor(wsc[:], in0=sg[:], scalar=SCALING, in1=mask[:],
                                       op0=OP.mult, op1=OP.mult)

        # ---- phase B: out.T[ot] = W[ot]^T @ x.T (+ B[ot]^T @ low_w.T) ----
        open_chains = []  # (ot, psum tile) awaiting their lora-B append

        def finish(ot, pb):
            nc.tensor.matmul(pb[:], lhsT=bT_sb[:, ot * 128:(ot + 1) * 128],
                             rhs=lowT_sb[:], start=False, stop=True)
            o_sb = outp.tile([128, T], BF16, tag="o", name=f"o{ot}")
            if ot >= OTN - 2:
                # split the critical last tile so its out-DMA overlaps the
                # second half's bias add
                h = T // 2
                nc.vector.tensor_scalar(o_sb[:, :h], pb[:, :h],
                                        scalar1=bias_sb[:, ot:ot + 1],
                                        scalar2=None, op0=OP.add)
                nc.sync.dma_start(out=out[:, ot, :h], in_=o_sb[:, :h])
                nc.vector.tensor_scalar(o_sb[:, h:], pb[:, h:],
                                        scalar1=bias_sb[:, ot:ot + 1],
                                        scalar2=None, op0=OP.add)
                nc.sync.dma_start(out=out[:, ot, h:], in_=o_sb[:, h:])
                return
            nc.vector.tensor_scalar(o_sb[:], pb[:],
                                    scalar1=bias_sb[:, ot:ot + 1], scalar2=None,
                                    op0=OP.add)
            nc.sync.dma_start(out=out[:, ot, :], in_=o_sb[:])

        lowT_sb = gw.tile([ER, T], BF16, tag="lowT")
        for ot in range(OTN):
            if ot in w_tiles:
                wq = w_tiles.pop(ot)
            else:
                wq = wp.tile([128, KT, 128], BF16, tag="w", name=f"w{ot}")
                eng = nc.scalar if ot % 2 == 0 else nc.sync
                eng.dma_start(out=wq[:], in_=wT[:, ot])
            pb = psB.tile([128, T], F32, tag="pb", name=f"pb{ot}")
            open_chains.append((ot, pb))
            for k in range(KT):
                nc.tensor.matmul(pb[:], lhsT=wq[:, k, :], rhs=x_k(k),
                                 start=(k == 0), stop=False)
                if k == 3 and ot >= 4:
                    # earlier chains' lora-B appends go mid-chain: their
                    # weight loads hide under this chain's matmul stream
                    # instead of bubbling at the chain boundary
                    target = {4: 4, 5: 3, 6: 2}.get(ot, 1)
                    while len(open_chains) > target:
                        finish(*open_chains.pop(0))
            if ot == 2:
                # replicate expert weights over their 16 ranks via tiny
                # matmul, then fold into the rank-space activations;
                # emitted late enough that the gating chain is done
                nc.tensor.matmul(wrep_ps[:], lhsT=Rm_sb[:], rhs=wsc[:],
                                 start=True, stop=True)
                wrep_sb = gw.tile([ER, T], F32, tag="wrepsb")
                nc.scalar.copy(wrep_sb[:], wrep_ps[:])
                nc.vector.tensor_tensor(lowT_sb[:], low_ps[:], wrep_sb[:],
                                        op=OP.mult)
        for ot, pb in open_chains:
            finish(ot, pb)


def build_module(debug=False):
    nc = bacc.Bacc("TRN2", target_bir_lowering=False, debug=debug)
    xT = nc.dram_tensor("xT", [128, KT, T], BF16, kind="ExternalInput")
    wT = nc.dram_tensor("wT", [128, OTN, KT, 128], BF16, kind="ExternalInput")
    aT = nc.dram_tensor("aT", [128, KT, ER], BF16, kind="ExternalInput")
    gT = nc.dram_tensor("gT", [128, KT, E], BF16, kind="ExternalInput")
    bT = nc.dram_tensor("bT", [ER, O], BF16, kind="ExternalInput")
    bias2 = nc.dram_tensor("bias2", [128, OTN], F32, kind="ExternalInput")
    Rm = nc.dram_tensor("Rm", [E, ER], BF16, kind="ExternalInput")
    out = nc.dram_tensor("out", [128, OTN, T], BF16, kind="ExternalOutput")
    with tile.TileContext(nc) as tc:
        build_body(nc, tc, (xT, wT, aT, gT, bT, bias2, Rm, out))
    nc.compile()
    return nc


def shard_inputs(x, gate_w, base_w, base_b, lora_A, lora_B):
    """FULL inputs -> list of 8 per-core input maps (host-side, free)."""
    x = np.asarray(x, dtype=np.float32)
    gate_w = np.asarray(gate_w, dtype=np.float32)
    base_w = np.asarray(base_w, dtype=np.float32)
    base_b = np.asarray(base_b, dtype=np.float32)
    lora_A = np.asarray(lora_A, dtype=np.float32)
    lora_B = np.asarray(lora_B, dtype=np.float32)

    xf = x.reshape(B * S, D)
    # replicated tensors (shared across cores)
    gT = np.ascontiguousarray(
        gate_w.T.reshape(KT, 128, E).transpose(1, 0, 2)).astype(NP_BF16)
    A_flat = lora_A.reshape(ER, D)
    aT = np.ascontiguousarray(
        A_flat.T.reshape(KT, 128, ER).transpose(1, 0, 2)).astype(NP_BF16)
    bT = np.ascontiguousarray(
        lora_B.transpose(0, 2, 1).reshape(ER, O)).astype(NP_BF16)
    Rm = np.repeat(np.eye(E, dtype=np.float32), R, axis=1).astype(NP_BF16)
    wT = np.ascontiguousarray(
        base_w.reshape(OTN, 128, KT, 128).transpose(3, 0, 2, 1)).astype(NP_BF16)
    bias2 = np.ascontiguousarray(base_b.reshape(OTN, 128).T)

    in_maps = []
    for c in range(N_CORES):
        x_c = xf[c * T:(c + 1) * T]                         # [T, D]
        xTc = np.ascontiguousarray(
            x_c.T.reshape(KT, 128, T).transpose(1, 0, 2)).astype(NP_BF16)
        in_maps.append({"xT": xTc, "wT": wT, "aT": aT, "gT": gT,
                        "bT": bT, "bias2": bias2, "Rm": Rm})
    return in_maps


def gather_outputs(results):
    """list of 8 per-core result maps -> FULL output [B, S, O]."""
    full = np.empty((B * S, O), dtype=np.float32)
    for c in range(N_CORES):
        oc = results[c]["out"]                              # [128, OTN, T] bf16
        full[c * T:(c + 1) * T, :] = \
            oc.transpose(2, 1, 0).reshape(T, O).astype(np.float32)
    return full.reshape(B, S, O)


_NC_CACHE = {}


def _get_module():
    if "nc" not in _NC_CACHE:
        _NC_CACHE["nc"] = build_module()
    return _NC_CACHE["nc"]


def run_sharded(in_maps, **run_kwargs):
    nc = _get_module()
    return run_bass_kernel_spmd(nc, in_maps, list(range(N_CORES)), **run_kwargs)


def kernel(x, gate_w, base_w, base_b, lora_A, lora_B):
    in_maps = shard_inputs(x, gate_w, base_w, base_b, lora_A, lora_B)
    res = run_sharded(in_maps)
    return gather_outputs(res.results)
